# revision 4
# baseline (speedup 1.0000x reference)
"""Trainium2 Bass kernel for nn_Agent (MLP -> LSTM(done-reset) -> actor/critic).

Strategy:
  - Data-parallel: B=1024 envs sharded 128/core over 8 cores.
  - Time-parallel on each core: T=4096 split into NBLK=32 blocks of L=128
    steps; each block is an independent chain warmed up W steps from zeros
    (exact because done=1 resets state; coverage verified at runtime on host,
    with an exact numpy patch fallback for any uncovered env/block).
  - V-layout: 32 chains arranged as partitions (4q x 2par x 16h = 128 rows)
    x free (4r x 128 envs = 512 cols). All ACT/DVE ops run [128, 512]-shaped
    at full 128-lane utilization.
  - Gates i,f,o,g computed into separate PSUM regions by 8 block-diagonal
    [128,128] weights-stationary matmuls (h-part + x-part accumulate).
  - done-masks broadcast to [128,512] by a constant PE matmul; applied as
    bf16 tensor-tensor multiplies to h and c at end of each slot.
  - Device outputs the full h history (bf16); the tiny actor/critic head
    (5x16 weights) and hN/cN replay run on host in fp32.
"""

import numpy as np
import ml_dtypes

T, B, H, XIN, A = 4096, 1024, 2, 16, 4  # placeholder; fixed below
T, B, XIN, H, A = 4096, 1024, 2, 16, 4
NC, E = 8, 128

_PROG_CACHE = {}
TRACE = False
LAST_RESULT = None
LAST_EXEC_WALL_NS = None


def _mappings(NBLK, L, W):
    S = L + W
    return S


def _host_build_consts(w1, b1, w_ih, w_hh, b_ih, b_hh):
    bf16 = ml_dtypes.bfloat16
    # gate type order: 0=i, 1=f, 2=o, 3=g ; reference rows order in w: i,f,g,o
    row = {0: 0, 1: 1, 2: 3, 3: 2}  # our tau -> block index in [i,f,g,o]
    gsth = np.zeros((4, 128, 128), np.float32)
    gstx = np.zeros((4, 128, 128), np.float32)
    I8 = np.eye(8, dtype=np.float32)
    for tau in range(4):
        rr = row[tau]
        U = w_hh[16 * rr:16 * rr + 16, :]      # [16(out k), 16(in k')]
        Wx = w_ih[16 * rr:16 * rr + 16, :]
        gsth[tau] = np.kron(I8, U.T)           # lhsT[k', m=k]
        gstx[tau] = np.kron(I8, Wx.T)
    mlpst = np.zeros((16, 128), np.float32)
    for g in range(8):
        # rows 2g + xi ; cols 16g + k ; value W1[k, xi]
        mlpst[2 * g:2 * g + 2, 16 * g:16 * g + 16] = w1.T
    mst = np.kron(I8, np.ones((1, 16), np.float32))  # [8,128]
    b4 = b_ih + b_hh
    biasv = np.zeros((128, 5), np.float32)
    for g in range(8):
        biasv[16 * g:16 * g + 16, 0] = b1
        for tau in range(4):
            biasv[16 * g:16 * g + 16, 1 + tau] = b4[16 * row[tau]:16 * row[tau] + 16]
    return (gsth.astype(bf16), gstx.astype(bf16), mlpst.astype(bf16),
            mst.astype(bf16), biasv)


def _build_program(Tc, NBLK, L, W):
    """Build the Bass program (one SPMD program for all cores)."""
    from contextlib import ExitStack
    import concourse.bass as bass
    import concourse.tile as tile
    from concourse import bacc, mybir

    bf = mybir.dt.bfloat16
    f32 = mybir.dt.float32
    S = L + W
    AF = mybir.ActivationFunctionType
    OP = mybir.AluOpType

    nc = bacc.Bacc("TRN2", target_bir_lowering=False, debug=False,
                   enable_asserts=False, num_devices=NC)
    xdm = nc.dram_tensor("xdm", [S, 16, 512], bf, kind="ExternalInput").ap()
    mdm = nc.dram_tensor("mdm", [S, 8, 512], bf, kind="ExternalInput").ap()
    gsth = nc.dram_tensor("gsth", [4, 128, 128], bf, kind="ExternalInput").ap()
    gstx = nc.dram_tensor("gstx", [4, 128, 128], bf, kind="ExternalInput").ap()
    mlpst = nc.dram_tensor("mlpst", [16, 128], bf, kind="ExternalInput").ap()
    mst = nc.dram_tensor("mst", [8, 128], bf, kind="ExternalInput").ap()
    biasv = nc.dram_tensor("biasv", [128, 5], f32, kind="ExternalInput").ap()
    h0m = nc.dram_tensor("h0m", [16, 128], bf, kind="ExternalInput").ap()
    c0m = nc.dram_tensor("c0m", [16, 128], bf, kind="ExternalInput").ap()
    huo = nc.dram_tensor("huo", [L, 128, 512], bf, kind="ExternalOutput").ap()

    with tile.TileContext(nc) as tc, ExitStack() as ctx:
        konst = ctx.enter_context(tc.tile_pool(name="konst", bufs=1))
        statep = ctx.enter_context(tc.tile_pool(name="state", bufs=1))
        xin = ctx.enter_context(tc.tile_pool(name="xin", bufs=3))
        work = ctx.enter_context(tc.tile_pool(name="work", bufs=2))
        hup = ctx.enter_context(tc.tile_pool(name="hup", bufs=3))
        psA = ctx.enter_context(tc.tile_pool(name="psA", bufs=1, space="PSUM"))
        psB = ctx.enter_context(tc.tile_pool(name="psB", bufs=2, space="PSUM"))

        gh_sb = konst.tile([128, 4 * 128], bf)
        gx_sb = konst.tile([128, 4 * 128], bf)
        mlp_sb = konst.tile([16, 128], bf)
        mst_sb = konst.tile([8, 128], bf)
        bias_sb = konst.tile([128, 5], f32)
        h0_sb = konst.tile([16, 128], bf)
        c0_sb = konst.tile([16, 128], bf)
        for tau in range(4):
            nc.sync.dma_start(gh_sb[:, 128 * tau:128 * tau + 128], gsth[tau, :, :])
            nc.sync.dma_start(gx_sb[:, 128 * tau:128 * tau + 128], gstx[tau, :, :])
        nc.sync.dma_start(mlp_sb[:], mlpst[:, :])
        nc.sync.dma_start(mst_sb[:], mst[:, :])
        nc.sync.dma_start(bias_sb[:], biasv[:, :])
        nc.sync.dma_start(h0_sb[:], h0m[:, :])
        nc.sync.dma_start(c0_sb[:], c0m[:, :])

        h_sb = statep.tile([128, 512], bf)
        c_sb = statep.tile([128, 512], bf)
        nc.vector.memset(h_sb[:], 0.0)
        nc.vector.memset(c_sb[:], 0.0)

        b1v = bias_sb[:, 0:1]

        for s in range(S):
            xt = xin.tile([16, 512], bf, tag="xt")
            nc.sync.dma_start(xt[:], xdm[s, :, :])
            mt = xin.tile([8, 512], bf, tag="mt")
            nc.sync.dma_start(mt[:], mdm[s, :, :])

            mlp_ps = psB.tile([128, 512], f32, tag="mlp")
            nc.tensor.matmul(mlp_ps[:], mlp_sb[:], xt[:], start=True, stop=True)
            hid = work.tile([128, 512], bf, tag="hid")
            nc.vector.tensor_scalar(hid[:], mlp_ps[:], b1v, 0.0,
                                    op0=OP.add, op1=OP.max)

            mm_ps = psB.tile([128, 512], f32, tag="mm")
            nc.tensor.matmul(mm_ps[:], mst_sb[:], mt[:], start=True, stop=True)
            mmb = work.tile([128, 512], bf, tag="mmb")
            nc.vector.tensor_scalar_mul(mmb[:], mm_ps[:], 1.0)

            ifo_ps = psA.tile([128, 1536], f32, tag="ifo")
            g_ps = psA.tile([128, 512], f32, tag="g")
            for tau in range(4):
                dst = g_ps[:] if tau == 3 else ifo_ps[:, 512 * tau:512 * tau + 512]
                nc.tensor.matmul(dst, gh_sb[:, 128 * tau:128 * tau + 128],
                                 h_sb[:], start=True, stop=False)
                nc.tensor.matmul(dst, gx_sb[:, 128 * tau:128 * tau + 128],
                                 hid[:], start=False, stop=True)

            ifo_t = work.tile([128, 1536], bf, tag="ifo_t")
            for tau in range(3):
                nc.scalar.activation(ifo_t[:, 512 * tau:512 * tau + 512],
                                     ifo_ps[:, 512 * tau:512 * tau + 512],
                                     AF.Sigmoid, bias=bias_sb[:, 1 + tau:2 + tau])
            g_t = work.tile([128, 512], bf, tag="g_t")
            nc.scalar.activation(g_t[:], g_ps[:], AF.Tanh, bias=bias_sb[:, 4:5])

            ig = work.tile([128, 512], bf, tag="ig")
            nc.vector.tensor_mul(ig[:], ifo_t[:, 0:512], g_t[:])
            nc.vector.tensor_mul(c_sb[:], ifo_t[:, 512:1024], c_sb[:])
            nc.vector.tensor_add(c_sb[:], c_sb[:], ig[:])
            tc_t = work.tile([128, 512], bf, tag="tc_t")
            nc.scalar.activation(tc_t[:], c_sb[:], AF.Tanh)
            hu = hup.tile([128, 512], bf, tag="hu")
            nc.vector.tensor_mul(hu[:], ifo_t[:, 1024:1536], tc_t[:])
            if s >= W:
                nc.sync.dma_start(huo[s - W, :, :], hu[:])
            nc.vector.tensor_mul(h_sb[:], hu[:], mmb[:])
            nc.vector.tensor_mul(c_sb[:], c_sb[:], mmb[:])

            if s == W - 1:
                nc.sync.dma_start(h_sb[0:16, 0:128], h0m[:, :])
                nc.sync.dma_start(c_sb[0:16, 0:128], c0m[:, :])

    nc.compile()
    return nc


def _host_pack(x, done, NBLK, L, W, Tc):
    """Build xdm [NC][S,16,512] and mdm [NC][S,8,512] (bf16)."""
    bf16 = ml_dtypes.bfloat16
    S = L + W
    TP = Tc + W + S  # padded length with slack
    xr = x.reshape(Tc, B, XIN)
    dr = done.reshape(Tc, B)
    x_pad = np.zeros((TP, B, XIN), np.float32)
    x_pad[W:W + Tc] = xr
    m_pad = np.zeros((TP, B), np.float32)  # mask = 1-done; pad done=1 -> m=0
    m_pad[W:W + Tc] = 1.0 - dr
    m_pad[W + Tc:] = 1.0  # beyond T: no reset

    # t-index grids
    c = np.arange(NBLK)                      # chains
    s = np.arange(S)
    tp = c[:, None] * L + s[None, :]         # [NBLK, S] padded index of t(c,s)
    # xdm[s, 2g+xi, 128r+e]  with g=2q+par, chain c = 8q+4par+r
    q, par, r = c // 8, (c // 4) % 2, c % 4
    g = 2 * q + par

    xga = x_pad[tp]                                    # [NBLK,S,B,2]
    mga = m_pad[tp + 1]                                # [NBLK,S,B]
    xdm = np.zeros((NC, S, 16, 512), np.float32)
    mdm = np.zeros((NC, S, 8, 512), np.float32)
    for core in range(NC):
        env = np.arange(128) + 128 * core
        xg = xga[:, :, env, :]
        mg = mga[:, :, env]
        for ci in range(NBLK):
            for xi in range(XIN):
                xdm[core, :, 2 * g[ci] + xi, 128 * r[ci]:128 * r[ci] + 128] = xg[ci, :, :, xi]
            mdm[core, :, g[ci], 128 * r[ci]:128 * r[ci] + 128] = mg[ci]
    return xdm.astype(bf16), mdm.astype(bf16)


def _host_unpack_hidden(hu_all, NBLK, L, Tc):
    """hu_all [NC][L,128,512] -> hidden [Tc, B, H] fp32."""
    hu = np.stack([np.asarray(h, dtype=np.float32) for h in hu_all])  # [NC,L,128,512]
    arr = hu.reshape(NC, L, 4, 2, 16, 4, 128)       # core, sl, q, par, k, r, e
    # -> [q, par, r, sl, core, e, k] ; chain index c = 8q+4par+r (q-major order)
    arr = arr.transpose(2, 3, 5, 1, 0, 6, 4)         # [4,2,4,L,NC,128,16]
    hidden = arr.reshape(NBLK * L, NC * 128, 16)
    return hidden[:Tc]


def _numpy_lstm(xh, m, h, c, w_ih, w_hh, b):
    """One step batched: xh [n,16] hid, m [n] mask, h/c [n,16]."""
    h = h * m[:, None]
    c = c * m[:, None]
    gates = xh @ w_ih.T + h @ w_hh.T + b
    i, f, g, o = np.split(gates, 4, axis=-1)
    i = 1 / (1 + np.exp(-i)); f = 1 / (1 + np.exp(-f)); o = 1 / (1 + np.exp(-o))
    g = np.tanh(g)
    c = f * c + i * g
    h = o * np.tanh(c)
    return h, c


def kernel(x, done, action, h0, c0, w1, b1, w_ih, w_hh, b_ih, b_hh,
           actor_w, actor_b, critic_w, critic_b):
    from concourse.bass_utils import run_bass_kernel_spmd

    x = np.asarray(x, np.float32); done = np.asarray(done)
    action = np.asarray(action)
    h0 = np.asarray(h0, np.float32); c0 = np.asarray(c0, np.float32)
    w1 = np.asarray(w1, np.float32); b1 = np.asarray(b1, np.float32)
    w_ih = np.asarray(w_ih, np.float32); w_hh = np.asarray(w_hh, np.float32)
    b_ih = np.asarray(b_ih, np.float32); b_hh = np.asarray(b_hh, np.float32)
    actor_w = np.asarray(actor_w, np.float32); actor_b = np.asarray(actor_b, np.float32)
    critic_w = np.asarray(critic_w, np.float32); critic_b = np.asarray(critic_b, np.float32)

    Tc = x.shape[0] // B
    NBLK, L, W = 32, Tc // 32, 26
    S = L + W
    bf16 = ml_dtypes.bfloat16

    key = (Tc, NBLK, L, W)
    if key not in _PROG_CACHE:
        _PROG_CACHE[key] = _build_program(Tc, NBLK, L, W)
    nc = _PROG_CACHE[key]

    gsth, gstx, mlpst, mst, biasv = _host_build_consts(w1, b1, w_ih, w_hh, b_ih, b_hh)
    xdm, mdm = _host_pack(x, done, NBLK, L, W, Tc)
    m0 = (1.0 - done.reshape(Tc, B)[0]).astype(np.float32)

    in_maps = []
    for core in range(NC):
        env = np.arange(128) + 128 * core
        h0m = (h0[env] * m0[env, None]).T.astype(bf16).copy()
        c0m = (c0[env] * m0[env, None]).T.astype(bf16).copy()
        in_maps.append({
            "xdm": xdm[core], "mdm": mdm[core],
            "gsth": gsth, "gstx": gstx, "mlpst": mlpst, "mst": mst,
            "biasv": biasv, "h0m": h0m, "c0m": c0m,
        })

    global LAST_RESULT, LAST_EXEC_WALL_NS
    import time as _time
    _t0 = _time.perf_counter()
    res = run_bass_kernel_spmd(nc, in_maps, core_ids=list(range(NC)),
                               trace=TRACE)
    LAST_EXEC_WALL_NS = int((_time.perf_counter() - _t0) * 1e9)
    LAST_RESULT = res
    hu_all = [res.results[i]["huo"] for i in range(NC)]
    hidden = _host_unpack_hidden(hu_all, NBLK, L, Tc)   # [Tc, B, 16] fp32

    # --- hN/cN: exact fp32 replay of the final block from zeros (relies on
    # the same done-coverage as the device's chain-31; patched below if needed)
    dr = done.reshape(Tc, B).astype(np.float32)
    b4 = b_ih + b_hh
    t0 = max((NBLK - 1) * L - W, 0)
    xh_tail = np.maximum(x.reshape(Tc, B, XIN)[t0:] @ w1.T + b1, 0.0)
    hh = np.zeros((B, H), np.float32); cc = np.zeros((B, H), np.float32)
    for i, t in enumerate(range(t0, Tc)):
        hh, cc = _numpy_lstm(xh_tail[i], 1.0 - dr[t], hh, cc, w_ih, w_hh, b4)
    hN, cN = hh, cc

    # --- exactness patch: any (chain>=1, env) without a done in its warmup
    # window gets an exact full-sequence numpy recompute.
    bad_envs = set()
    for c in range(1, NBLK):
        lo, hi = c * L - W, c * L
        cov = dr[max(lo, 0):hi].any(axis=0)
        for e in np.nonzero(cov == 0)[0]:
            bad_envs.add(int(e))
    if bad_envs:
        xh_full = np.maximum(x.reshape(Tc, B, XIN) @ w1.T + b1, 0.0)
        for e in sorted(bad_envs):
            hh1 = h0[e:e + 1].copy(); cc1 = c0[e:e + 1].copy()
            for t in range(Tc):
                hh1, cc1 = _numpy_lstm(xh_full[t, e:e + 1], 1.0 - dr[t, e:e + 1],
                                       hh1, cc1, w_ih, w_hh, b4)
                hidden[t, e] = hh1[0]
            hN[e] = hh1[0]; cN[e] = cc1[0]

    # --- head (host, fp32) ---
    hid2 = hidden.reshape(-1, H)
    logits = hid2 @ actor_w.T + actor_b
    mx = logits.max(axis=-1, keepdims=True)
    lse = mx + np.log(np.exp(logits - mx).sum(axis=-1, keepdims=True))
    logp_all = logits - lse
    logp = np.take_along_axis(logp_all, action.reshape(-1, 1).astype(np.int64),
                              axis=1)[:, 0]
    entropy = -(np.exp(logp_all) * logp_all).sum(axis=-1)
    value = (hid2 @ critic_w.T + critic_b)[:, 0]

    return (action, logp.astype(np.float32), entropy.astype(np.float32),
            value.astype(np.float32), hN.astype(np.float32), cN.astype(np.float32))
